# revision 44
# baseline (speedup 1.0000x reference)
"""Trainium2 Bass kernel: 2-layer transformer encoder (query-axis softmax,
batch-moments normalization), data-parallel over batch across 8 NeuronCores.

Layout strategy: all on-device activations are feature-major [d, s] so every
matmul contraction runs over the 128-partition axis with no device-side
transposes. The host pre-adds the positional encoding, pre-transposes X, and
pre-packs all weights into the SBUF layouts the TensorEngine wants. Batch-norm
moments (sum, sum-of-squares over the batch axis) are AllReduced across the 8
cores. Matmuls run in bf16 with fp32 PSUM accumulation; the residual stream
and all statistics stay fp32.
"""

import os
import sys

import numpy as np

sys.path.insert(0, "/opt/trn_rl_repo")

import ml_dtypes

B, S, E, DK, H, D2 = 32, 512, 512, 128, 6, 1024
EPS = 1e-3
N_CORES = 8
BPC = B // N_CORES  # batch elements per core
TCN = S // 128      # t-chunks (key positions)
EC1 = E // 128      # feature chunks, layer-1 input width
EC2 = D2 // 128     # feature chunks, layer-2 input width
DC = D2 // 128      # output-feature chunks
SCALE = float(1.0 / np.sqrt(DK))

_PROGRAM_CACHE = {}


def _pos_encoding():
    i = np.arange(S, dtype=np.float64)[:, None]
    j = np.arange(0, E, 2)[None, :]
    pe = np.zeros((S, E), dtype=np.float64)
    pe[:, 0::2] = np.sin(i / 10000.0 ** ((2 * j) // E))
    pe[:, 1::2] = np.cos(i / 10000.0 ** ((2 * (j + 1)) // E))
    return pe.astype(np.float32)


def _build(with_bias, repeat=1, timing=False, ablate=()):
    """Build + compile the SPMD program. with_bias enables the rarely-needed
    bias adds (the reference generates all-zero biases). repeat>1 emits the
    whole computation repeat times back-to-back; timing=True makes all inputs
    device-Internal (garbage data) and shrinks the output so the RPC transfer
    floor vanishes (timing harness only)."""
    import concourse.bacc as bacc
    import concourse.mybir as mybir
    import concourse.tile as tile

    f32 = mybir.dt.float32
    bf16 = mybir.dt.bfloat16
    AF = mybir.ActivationFunctionType

    nc = bacc.Bacc("TRN2", target_bir_lowering=False, debug=False,
                   num_devices=N_CORES)

    def din(name, shape, dt):
        kind = "Internal" if timing else "ExternalInput"
        return nc.dram_tensor(name, list(shape), dt, kind=kind)

    xpt_d = din("xpt", (BPC, EC1, 128, S), f32)
    xpt16_d = din("xpt16", (BPC, EC1, 128, S), bf16)
    wq1_d = din("wq1", (EC1, 128, H * DK), bf16)
    wk1_d = din("wk1", (EC1, 128, H * DK), bf16)
    wv1_d = din("wv1", (EC1, 128, H * DK), bf16)
    wo1_d = din("wo1", (H, 128, D2), bf16)
    wff1_d = din("wff1", (DC, 128, D2), bf16)
    bff1_d = din("bff1", (128, DC), f32)
    wq2_d = din("wq2", (EC2, 128, H * DK), bf16)
    wk2_d = din("wk2", (EC2, 128, H * DK), bf16)
    wv2_d = din("wv2", (EC2, 128, H * DK), bf16)
    wo2_d = din("wo2", (DC, 128, H * DK), bf16)  # d-chunk-sliced layout
    wff2_d = din("wff2", (DC, 128, D2), bf16)
    bff2_d = din("bff2", (128, DC), f32)
    if with_bias:
        bqk1_d = din("bqk1", (2, 128, H), f32)   # [q/k][dk][head]
        bv1_d = din("bv1", (1, H * DK), f32)
        bo1_d = din("bo1", (128, DC), f32)
        bqk2_d = din("bqk2", (2, 128, H), f32)
        bv2_d = din("bv2", (1, H * DK), f32)
        bo2_d = din("bo2", (128, DC), f32)
    if timing:
        out_d = nc.dram_tensor("out_big", [BPC, D2, S], f32, kind="Internal")
        outsm_d = nc.dram_tensor("out", [128, 512], f32,
                                 kind="ExternalOutput")
    else:
        out_d = nc.dram_tensor("out", [BPC, D2, S], f32,
                               kind="ExternalOutput")

    import concourse.bass as bass
    from contextlib import ExitStack

    with tile.TileContext(nc) as tc:
        with ExitStack() as ctx:
            ep = ctx.enter_context
            p_y = ep(tc.tile_pool(name="p_y", bufs=1))
            p_stats = ep(tc.tile_pool(name="p_stats", bufs=1))

            p_e16 = ep(tc.tile_pool(name="p_e16", bufs=9))
            p_z16 = ep(tc.tile_pool(name="p_z16", bufs=8))
            p_kq = ep(tc.tile_pool(name="p_kq", bufs=6))
            p_vsl = ep(tc.tile_pool(name="p_vsl", bufs=8))
            p_v16 = ep(tc.tile_pool(name="p_v16", bufs=6))
            p_ms = ep(tc.tile_pool(name="p_ms", bufs=5))
            p_tanh = ep(tc.tile_pool(name="p_tanh", bufs=3))
            p_xh = ep(tc.tile_pool(name="p_xh", bufs=26))
            p_sqt = ep(tc.tile_pool(name="p_sqt", bufs=3))
            p_di = ep(tc.tile_pool(name="p_di", bufs=10))
            ps512 = ep(tc.tile_pool(name="ps512", bufs=6, space="PSUM"))
            ps256 = ep(tc.tile_pool(name="ps256", bufs=2, space="PSUM"))
            p_dram = ep(tc.tile_pool(name="p_dram", bufs=2, space="DRAM"))

            y_slabs = [
                p_y.tile([128, DC * S], f32, tag=f"y{b}", name=f"y{b}")
                for b in range(BPC)
            ]
            sum_slab = p_stats.tile([128, DC * S], f32, tag="sum", name="sum")
            sq_slab = p_stats.tile([128, DC * S], f32, tag="sq", name="sq")
            eps_t = p_stats.tile([128, 1], f32, tag="eps", name="eps")
            nc.vector.memset(eps_t, float(EPS))
            f16 = mybir.dt.float16
            fp16_ar = "fp32ar" not in ablate
            if fp16_ar:
                # fp16 staging for the AllReduce payload (halves wire bytes;
                # fp16's 10-bit mantissa keeps the moments accurate, unlike
                # bf16)
                st_sum = p_stats.tile([128, DC * S // 2], f16, tag="stsum",
                                      name="stsum")
                st_sq = p_stats.tile([128, DC * S // 2], f16, tag="stsq",
                                     name="stsq")

            def cs(c):
                return slice(c * S, (c + 1) * S)

            def attention(b, x16h, wq, wk, wv, wo_lhsT, resid, y_slab, ECn,
                          bqk=None, bv_bc=None, bo=None):
                # x16h: [ECn][2] SBUF tiles [128, SH] bf16, feature-major,
                # split by s-half so projections can start on half 0 while
                # half 1's batch-norm AllReduce is still in flight.
                v16 = []
                for t in range(TCN):
                    xh = t // 2
                    xs = (t % 2) * 128
                    vp1 = ps512.tile([128, 512], f32, tag="ps", name="vp1")
                    vp2 = ps256.tile([128, 256], f32, tag="ps2", name="vp2")
                    for ec in range(ECn):
                        nc.tensor.matmul(
                            vp1, lhsT=x16h[ec][xh][:, xs:xs + 128],
                            rhs=wv[:, ec, 0:512],
                            start=(ec == 0), stop=(ec == ECn - 1))
                    for ec in range(ECn):
                        nc.tensor.matmul(
                            vp2, lhsT=x16h[ec][xh][:, xs:xs + 128],
                            rhs=wv[:, ec, 512:768],
                            start=(ec == 0), stop=(ec == ECn - 1))
                    vt = p_v16.tile([128, H * DK], bf16, tag="v16",
                                    name=f"v16_{t}")
                    nc.vector.tensor_copy(out=vt[:, 0:512], in_=vp1)
                    nc.vector.tensor_copy(out=vt[:, 512:768], in_=vp2)
                    if bv_bc is not None:
                        nc.vector.tensor_add(out=vt, in0=vt, in1=bv_bc)
                    v16.append(vt)

                z16 = []
                for h in range(H):
                    hk = slice(h * DK, (h + 1) * DK)
                    kp = ps512.tile([128, 512], f32, tag="ps", name="kp")
                    qp = ps512.tile([128, 512], f32, tag="ps", name="qp")
                    for sh_ in (0, 1):
                        ssl = slice(sh_ * SH, (sh_ + 1) * SH)
                        for ec in range(ECn):
                            nc.tensor.matmul(
                                kp[:, ssl], lhsT=wk[:, ec, hk],
                                rhs=x16h[ec][sh_],
                                start=(ec == 0), stop=(ec == ECn - 1))
                        for ec in range(ECn):
                            nc.tensor.matmul(
                                qp[:, ssl], lhsT=wq[:, ec, hk],
                                rhs=x16h[ec][sh_],
                                start=(ec == 0), stop=(ec == ECn - 1))
                    k16 = p_kq.tile([128, S], bf16, tag="kq", name="k16")
                    q16 = p_kq.tile([128, S], bf16, tag="kq", name="q16")
                    if bqk is not None:
                        nc.vector.tensor_scalar_add(
                            out=k16, in0=kp, scalar1=bqk[1, :, h:h + 1])
                        nc.vector.tensor_scalar(
                            out=q16, in0=qp, scalar1=bqk[0, :, h:h + 1],
                            scalar2=SCALE, op0=mybir.AluOpType.add,
                            op1=mybir.AluOpType.mult)
                    else:
                        nc.vector.tensor_copy(out=k16, in_=kp)
                        nc.scalar.activation(out=q16, in_=qp, func=AF.Copy,
                                             scale=SCALE)
                    es = []
                    dsum4 = p_di.tile([128, TCN], f32, tag="dsum",
                                      name="dsum4")
                    for t in range(TCN):
                        sp = ps512.tile([128, 512], f32, tag="ps", name="sp")
                        nc.tensor.matmul(sp,
                                         lhsT=k16[:, t * 128:(t + 1) * 128],
                                         rhs=q16, start=True, stop=True)
                        e16 = p_e16.tile([128, S], bf16, tag="e16",
                                         name=f"e16_{t}")
                        nc.scalar.activation(out=e16, in_=sp, func=AF.Exp,
                                             accum_out=dsum4[:, t:t + 1])
                        es.append(e16)
                    dinv4 = p_di.tile([128, TCN], f32, tag="dinv",
                                      name="dinv4")
                    nc.vector.reciprocal(out=dinv4, in_=dsum4)
                    op_ = ps512.tile([128, 512], f32, tag="ps", name="op")
                    for t in range(TCN):
                        vsl = p_vsl.tile([128, DK], bf16, tag="vsl",
                                         name="vsl")
                        nc.vector.tensor_scalar_mul(out=vsl,
                                                    in0=v16[t][:, hk],
                                                    scalar1=dinv4[:, t:t + 1])
                        nc.tensor.matmul(op_, lhsT=vsl, rhs=es[t],
                                         start=(t == 0), stop=(t == TCN - 1))
                    zt = p_z16.tile([128, S], bf16, tag="z16", name=f"z16_{h}")
                    nc.scalar.activation(out=zt, in_=op_, func=AF.Copy)
                    z16.append(zt)

                for c in range(DC):
                    wp_ = ps512.tile([128, 512], f32, tag="ps", name="wp")
                    lhsT_list = wo_lhsT(c)
                    for h in range(H):
                        nc.tensor.matmul(wp_, lhsT=lhsT_list[h], rhs=z16[h],
                                         start=(h == 0), stop=(h == H - 1))
                    if bo is not None:
                        nc.vector.tensor_scalar_add(out=wp_, in0=wp_,
                                                    scalar1=bo[:, c:c + 1])
                    nc.vector.tensor_add(out=y_slab[:, cs(c)], in0=wp_,
                                         in1=resid(c))

            def make_n16h(b, tagpfx):
                # bf16 half-tiles of the normalized slab, per (chunk, s-half)
                n16h = []
                for c in range(DC):
                    pair = []
                    for hh in (0, 1):
                        t = p_xh.tile([128, SH], bf16, tag="x16h",
                                      name=f"{tagpfx}_{b}_{c}_{hh}")
                        o0 = c * S + hh * SH
                        nc.gpsimd.tensor_copy(out=t,
                                              in_=y_slabs[b][:, o0:o0 + SH])
                        pair.append(t)
                    n16h.append(pair)
                return n16h

            def ffn(b, wff, bff, half):
                # s-half-wise FFN: n16 slices, matmuls, tanh and residual add
                # all touch only this s-half, so half 0 runs while half 1's
                # batch-norm AllReduce is still in flight.
                n16 = []
                for ci in range(DC):
                    t = p_xh.tile([128, SH], bf16, tag="x16h",
                                  name=f"nh_{b}_{ci}")
                    o0 = ci * S + half * SH
                    nc.gpsimd.tensor_copy(out=t,
                                          in_=y_slabs[b][:, o0:o0 + SH])
                    n16.append(t)
                for co in range(DC):
                    fps = ps256.tile([128, SH], f32, tag="ps2", name="fps")
                    for ci in range(DC):
                        nc.tensor.matmul(
                            fps, lhsT=wff[:, ci, co * 128:(co + 1) * 128],
                            rhs=n16[ci], start=(ci == 0), stop=(ci == DC - 1))
                    th = p_tanh.tile([128, SH], f32, tag="tanh", name="th")
                    nc.scalar.activation(out=th, in_=fps, func=AF.Tanh,
                                         bias=bff[:, co:co + 1], scale=1.0)
                    o0 = co * S + half * SH
                    ysl = y_slabs[b][:, o0:o0 + SH]
                    nc.vector.tensor_add(out=ysl, in0=th, in1=ysl)

            SH = S // 2                              # s-half width (256)

            def shalf(slab, h):
                # strided view of an [128, DC*S] slab: s-half h of every chunk
                return slab[:].rearrange("p (c s) -> p c s",
                                         c=DC)[:, :, h * SH:(h + 1) * SH]

            def bn(idx):
                # Stats are accumulated over the whole slab, then the
                # AllReduce + finalize + normalize are split over two
                # s-halves: everything after the first half's AllReduce
                # (normalize + the next phase's s-half-0 compute) overlaps
                # the second half's AllReduce.
                if "nobn" in ablate:
                    return
                for b in range(BPC):
                    if b == 0:
                        nc.vector.tensor_copy(out=sum_slab, in_=y_slabs[0])
                    else:
                        nc.vector.tensor_add(out=sum_slab, in0=sum_slab,
                                             in1=y_slabs[b])
                    for c in range(DC):
                        yc = y_slabs[b][:, cs(c)]
                        if b == 0:
                            nc.scalar.activation(out=sq_slab[:, cs(c)],
                                                 in_=yc, func=AF.Square)
                        else:
                            tmp = p_sqt.tile([128, S], f32, tag="sqt",
                                             name="sqt")
                            nc.scalar.activation(out=tmp, in_=yc,
                                                 func=AF.Square)
                            nc.vector.tensor_add(out=sq_slab[:, cs(c)],
                                                 in0=sq_slab[:, cs(c)],
                                                 in1=tmp)
                HS = DC * SH                         # per-half payload (2048)
                for half in (0, 1):
                    sums = shalf(sum_slab, half)
                    sqs = shalf(sq_slab, half)
                    ardt = f16 if fp16_ar else f32
                    arin = p_dram.tile([128, 2 * HS], ardt, tag=f"arin{half}",
                                       name=f"arin{idx}_{half}")
                    arout = p_dram.tile([128, 2 * HS], ardt,
                                        tag=f"arout{half}",
                                        name=f"arout{idx}_{half}",
                                        addr_space="Shared")
                    if fp16_ar:
                        stsum_v = st_sum[:].rearrange("p (c s) -> p c s",
                                                      c=DC)
                        stsq_v = st_sq[:].rearrange("p (c s) -> p c s", c=DC)
                        nc.vector.tensor_copy(out=stsum_v, in_=sums)
                        nc.vector.tensor_copy(out=stsq_v, in_=sqs)
                        src_sum, src_sq = st_sum[:], st_sq[:]
                    else:
                        src_sum, src_sq = sums, sqs
                    # chunked bounce DMAs parallelize across DMA queues
                    for i in range(2):
                        qn = HS // 2
                        if fp16_ar:
                            nc.sync.dma_start(
                                out=arin[:, i * qn:(i + 1) * qn],
                                in_=st_sum[:, i * qn:(i + 1) * qn])
                            nc.sync.dma_start(
                                out=arin[:, HS + i * qn:HS + (i + 1) * qn],
                                in_=st_sq[:, i * qn:(i + 1) * qn])
                        else:
                            csl = slice(i * DC // 2, (i + 1) * DC // 2)
                            nc.sync.dma_start(
                                out=arin[:, i * qn:(i + 1) * qn].rearrange(
                                    "p (c s) -> p c s", c=DC // 2),
                                in_=sums[:, csl, :])
                            nc.sync.dma_start(
                                out=arin[:, HS + i * qn:HS + (i + 1) * qn]
                                .rearrange("p (c s) -> p c s", c=DC // 2),
                                in_=sqs[:, csl, :])
                    if "noar" in ablate:
                        nc.sync.dma_start(out=arout, in_=arin)
                    else:
                        nc.gpsimd.collective_compute(
                            "AllReduce", mybir.AluOpType.add,
                            replica_groups=[list(range(N_CORES))],
                            ins=[arin.opt()], outs=[arout.opt()])
                    for i in range(2):
                        qn = HS // 2
                        if fp16_ar:
                            nc.sync.dma_start(
                                out=st_sum[:, i * qn:(i + 1) * qn],
                                in_=arout[:, i * qn:(i + 1) * qn])
                            nc.sync.dma_start(
                                out=st_sq[:, i * qn:(i + 1) * qn],
                                in_=arout[:, HS + i * qn:HS + (i + 1) * qn])
                        else:
                            csl = slice(i * DC // 2, (i + 1) * DC // 2)
                            nc.sync.dma_start(
                                out=sums[:, csl, :],
                                in_=arout[:, i * qn:(i + 1) * qn]
                                .rearrange("p (c s) -> p c s", c=DC // 2))
                            nc.sync.dma_start(
                                out=sqs[:, csl, :],
                                in_=arout[:, HS + i * qn:HS + (i + 1) * qn]
                                .rearrange("p (c s) -> p c s", c=DC // 2))
                    # finalize in place: sum half -> mean, sq half -> rstd
                    if fp16_ar:
                        nc.vector.tensor_scalar_mul(out=sums, in0=stsum_v,
                                                    scalar1=1.0 / B)
                        nc.vector.tensor_scalar_mul(out=sqs, in0=stsq_v,
                                                    scalar1=1.0 / B)
                    else:
                        nc.vector.tensor_scalar_mul(out=sums, in0=sums,
                                                    scalar1=1.0 / B)
                        nc.vector.tensor_scalar_mul(out=sqs, in0=sqs,
                                                    scalar1=1.0 / B)
                    for c in range(DC):
                        hs0 = c * S + half * SH
                        msq = p_ms.tile([128, SH], f32, tag="ms", name="msq")
                        nc.vector.tensor_mul(
                            out=msq, in0=sum_slab[:, hs0:hs0 + SH],
                            in1=sum_slab[:, hs0:hs0 + SH])
                        nc.vector.tensor_sub(
                            out=sq_slab[:, hs0:hs0 + SH],
                            in0=sq_slab[:, hs0:hs0 + SH], in1=msq)
                    nc.scalar.activation(out=sqs, in_=sqs, func=AF.Sqrt,
                                         bias=eps_t)
                    nc.vector.reciprocal(out=sqs, in_=sqs)
                    for b in range(BPC):
                        ysl = shalf(y_slabs[b], half)
                        nc.vector.tensor_sub(out=ysl, in0=ysl, in1=sums)
                        nc.vector.tensor_mul(out=ysl, in0=ysl, in1=sqs)

            def l1_attention():
                with tc.tile_pool(name="wl1", bufs=1) as wp, \
                     tc.tile_pool(name="p_xpf", bufs=5) as p_xpf:
                    wq = wp.tile([128, EC1, H * DK], bf16, name="wq1s")
                    wk = wp.tile([128, EC1, H * DK], bf16, name="wk1s")
                    wv = wp.tile([128, EC1, H * DK], bf16, name="wv1s")
                    wo = wp.tile([128, H, D2], bf16, name="wo1s")
                    nc.sync.dma_start(
                        out=wq, in_=wq1_d.ap().rearrange("c p k -> p c k"))
                    nc.sync.dma_start(
                        out=wk, in_=wk1_d.ap().rearrange("c p k -> p c k"))
                    nc.sync.dma_start(
                        out=wv, in_=wv1_d.ap().rearrange("c p k -> p c k"))
                    nc.sync.dma_start(
                        out=wo, in_=wo1_d.ap().rearrange("h p d -> p h d"))
                    bqk1 = bv1_bc = bo1 = None
                    if with_bias:
                        bqk1 = wp.tile([2, 128, H], f32, name="bqk1s")
                        nc.sync.dma_start(out=bqk1, in_=bqk1_d.ap())
                        bv1t = wp.tile([1, H * DK], f32, name="bv1s")
                        nc.sync.dma_start(out=bv1t, in_=bv1_d.ap())
                        bv1_bc = bass.AP(
                            tensor=bv1t.tensor, offset=bv1t.offset,
                            ap=[[0, 128], [1, H * DK]])
                        bo1 = wp.tile([128, DC], f32, name="bo1s")
                        nc.sync.dma_start(out=bo1, in_=bo1_d.ap())

                    def wo_lhsT1(c):
                        return [wo[:, h, c * 128:(c + 1) * 128]
                                for h in range(H)]

                    for b in range(BPC):
                        x16h = []
                        xf4 = []
                        for c in range(EC1):
                            pair = []
                            for hh in (0, 1):
                                t = p_xh.tile([128, SH], bf16, tag="x16h",
                                              name=f"x16_{b}_{c}_{hh}")
                                nc.sync.dma_start(
                                    out=t,
                                    in_=xpt16_d.ap()[b, c][:, hh * SH:
                                                           (hh + 1) * SH])
                                pair.append(t)
                            x16h.append(pair)
                            xf = p_xpf.tile([128, S], f32, tag="xpf",
                                            name=f"xpf_{c}")
                            nc.sync.dma_start(out=xf, in_=xpt_d.ap()[b, c])
                            xf4.append(xf)

                        def resid1(c, xf4=xf4):
                            return xf4[c % EC1]

                        attention(b, x16h, wq, wk, wv, wo_lhsT1, resid1,
                                  y_slabs[b], EC1, bqk=bqk1, bv_bc=bv1_bc,
                                  bo=bo1)

            def ffn_phase(wff_d, bff_d, pname):
                with tc.tile_pool(name=pname, bufs=1) as wp:
                    wff = wp.tile([128, DC, D2], bf16, name=f"{pname}w")
                    nc.sync.dma_start(
                        out=wff, in_=wff_d.ap().rearrange("c p d -> p c d"))
                    bff = wp.tile([128, DC], f32, name=f"{pname}b")
                    nc.sync.dma_start(out=bff, in_=bff_d.ap())
                    for half in (0, 1):
                        for b in range(BPC):
                            ffn(b, wff, bff, half)

            def l2_attention():
                with tc.tile_pool(name="wl2", bufs=1) as wp, \
                     tc.tile_pool(name="wo2p", bufs=3) as wo2p:
                    wq2 = wp.tile([128, EC2, H * DK], bf16, name="wq2s")
                    wk2 = wp.tile([128, EC2, H * DK], bf16, name="wk2s")
                    wv2 = wp.tile([128, EC2, H * DK], bf16, name="wv2s")
                    nc.sync.dma_start(
                        out=wq2, in_=wq2_d.ap().rearrange("c p k -> p c k"))
                    nc.sync.dma_start(
                        out=wk2, in_=wk2_d.ap().rearrange("c p k -> p c k"))
                    nc.sync.dma_start(
                        out=wv2, in_=wv2_d.ap().rearrange("c p k -> p c k"))
                    bqk2 = bv2_bc = bo2 = None
                    if with_bias:
                        bqk2 = wp.tile([2, 128, H], f32, name="bqk2s")
                        nc.sync.dma_start(out=bqk2, in_=bqk2_d.ap())
                        bv2t = wp.tile([1, H * DK], f32, name="bv2s")
                        nc.sync.dma_start(out=bv2t, in_=bv2_d.ap())
                        bv2_bc = bass.AP(
                            tensor=bv2t.tensor, offset=bv2t.offset,
                            ap=[[0, 128], [1, H * DK]])
                        bo2 = wp.tile([128, DC], f32, name="bo2s")
                        nc.sync.dma_start(out=bo2, in_=bo2_d.ap())

                    for b in range(BPC):
                        n16h = make_n16h(b, "na")

                        def wo_lhsT2(c):
                            w = wo2p.tile([128, H * DK], bf16, tag="wo2s",
                                          name="wo2s")
                            nc.sync.dma_start(out=w, in_=wo2_d.ap()[c])
                            return [w[:, h * DK:(h + 1) * DK]
                                    for h in range(H)]

                        def resid2(c, b=b):
                            return y_slabs[b][:, cs(c)]

                        attention(b, n16h, wq2, wk2, wv2, wo_lhsT2, resid2,
                                  y_slabs[b], EC2, bqk=bqk2, bv_bc=bv2_bc,
                                  bo=bo2)

            for _rep in range(repeat):
                if "noattn" not in ablate:
                    l1_attention()
                bn(0)
                if "noffn" not in ablate:
                    ffn_phase(wff1_d, bff1_d, "wf1")
                bn(1)
                if "noattn" not in ablate:
                    l2_attention()
                bn(2)
                if "noffn" not in ablate:
                    ffn_phase(wff2_d, bff2_d, "wf2")
                bn(3)

            # ---------------- Output ----------------
            for half in (0, 1):
                for b in range(BPC):
                    nc.sync.dma_start(
                        out=out_d.ap()[b].rearrange("(c p) s -> p c s",
                                                    p=128)
                        [:, :, half * SH:(half + 1) * SH],
                        in_=shalf(y_slabs[b], half))
            if timing:
                nc.sync.dma_start(out=outsm_d.ap(),
                                  in_=y_slabs[0][:, 0:512])

    nc.compile()
    return nc


def _get_program(with_bias):
    key = bool(with_bias)
    if key not in _PROGRAM_CACHE:
        _PROGRAM_CACHE[key] = _build(key)
    return _PROGRAM_CACHE[key]


def _pack_qkv(w):
    # (H, din, DK) -> (din//128, 128, H*DK)
    din = w.shape[1]
    return np.ascontiguousarray(
        w.transpose(1, 0, 2).reshape(din, H * DK).reshape(din // 128, 128,
                                                          H * DK))


def _prep_in_maps(inputs):
    bf = ml_dtypes.bfloat16
    f32 = np.float32
    g = lambda n: np.asarray(inputs[n], f32)

    X = g("X")
    pos = _pos_encoding()
    xp = (X + pos[None]).astype(f32)                       # (B, S, E)
    xpT = np.ascontiguousarray(xp.transpose(0, 2, 1))      # (B, E, S)
    xpt = xpT.reshape(B, EC1, 128, S)
    xpt16 = xpt.astype(bf)

    wq1 = _pack_qkv(g("Wq1")).astype(bf)
    wk1 = _pack_qkv(g("Wk1")).astype(bf)
    wv1 = _pack_qkv(g("Wv1")).astype(bf)
    wo1 = np.ascontiguousarray(g("Wo1").reshape(H, 128, D2)).astype(bf)
    wff1 = np.ascontiguousarray(g("Wff1").reshape(DC, 128, D2)).astype(bf)
    bff1 = np.ascontiguousarray(g("bff1").reshape(DC, 128).T).astype(f32)
    wq2 = _pack_qkv(g("Wq2")).astype(bf)
    wk2 = _pack_qkv(g("Wk2")).astype(bf)
    wv2 = _pack_qkv(g("Wv2")).astype(bf)
    # Wo2 sliced by output d-chunk: wo2[c, p, h*DK+m] = Wo2[h*DK+p, c*128+m]
    wo2 = np.ascontiguousarray(
        g("Wo2").reshape(H, DK, DC, 128).transpose(2, 1, 0, 3)
        .reshape(DC, 128, H * DK)).astype(bf)
    wff2 = np.ascontiguousarray(g("Wff2").reshape(DC, 128, D2)).astype(bf)
    bff2 = np.ascontiguousarray(g("bff2").reshape(DC, 128).T).astype(f32)

    zb = [g(n) for n in ("bq1", "bk1", "bv1", "bo1", "bq2", "bk2", "bv2",
                         "bo2")]
    with_bias = any(np.any(z) for z in zb)

    shared = dict(wq1=wq1, wk1=wk1, wv1=wv1, wo1=wo1, wff1=wff1, bff1=bff1,
                  wq2=wq2, wk2=wk2, wv2=wv2, wo2=wo2, wff2=wff2, bff2=bff2)
    if with_bias:
        bq1, bk1, bv1, bo1, bq2, bk2, bv2, bo2 = zb
        shared.update(
            bqk1=np.stack([bq1.T, bk1.T]).astype(f32),
            bv1=bv1.reshape(1, H * DK).astype(f32),
            bo1=np.ascontiguousarray(bo1.reshape(DC, 128).T).astype(f32),
            bqk2=np.stack([bq2.T, bk2.T]).astype(f32),
            bv2=bv2.reshape(1, H * DK).astype(f32),
            bo2=np.ascontiguousarray(bo2.reshape(DC, 128).T).astype(f32),
        )

    in_maps = []
    for core in range(N_CORES):
        m = dict(shared)
        m["xpt"] = np.ascontiguousarray(xpt[core * BPC:(core + 1) * BPC])
        m["xpt16"] = np.ascontiguousarray(xpt16[core * BPC:(core + 1) * BPC])
        in_maps.append(m)
    return in_maps, with_bias


def kernel(**inputs):
    in_maps, with_bias = _prep_in_maps(inputs)
    nc = _get_program(with_bias)
    from concourse import bass_utils
    res = bass_utils.run_bass_kernel_spmd(nc, in_maps,
                                          core_ids=list(range(N_CORES)))
    outT = np.concatenate([res.results[i]["out"] for i in range(N_CORES)],
                          axis=0)                          # (B, D2, S)
    return np.ascontiguousarray(outT.transpose(0, 2, 1))   # (B, S, D2) f32


# revision 45
# speedup vs baseline: 1.1922x; 1.1922x over previous
"""Trainium2 Bass kernel: 2-layer transformer encoder (query-axis softmax,
batch-moments normalization), data-parallel over batch across 8 NeuronCores.

Layout strategy: all on-device activations are feature-major [d, s] so every
matmul contraction runs over the 128-partition axis with no device-side
transposes. The host pre-adds the positional encoding, pre-transposes X, and
pre-packs all weights into the SBUF layouts the TensorEngine wants. Batch-norm
moments (sum, sum-of-squares over the batch axis) are AllReduced across the 8
cores. Matmuls run in bf16 with fp32 PSUM accumulation; the residual stream
and all statistics stay fp32.
"""

import os
import sys

import numpy as np

sys.path.insert(0, "/opt/trn_rl_repo")

import ml_dtypes

B, S, E, DK, H, D2 = 32, 512, 512, 128, 6, 1024
EPS = 1e-3
N_CORES = 8
BPC = B // N_CORES  # batch elements per core
TCN = S // 128      # t-chunks (key positions)
EC1 = E // 128      # feature chunks, layer-1 input width
EC2 = D2 // 128     # feature chunks, layer-2 input width
DC = D2 // 128      # output-feature chunks
SCALE = float(1.0 / np.sqrt(DK))

_PROGRAM_CACHE = {}


def _pos_encoding():
    i = np.arange(S, dtype=np.float64)[:, None]
    j = np.arange(0, E, 2)[None, :]
    pe = np.zeros((S, E), dtype=np.float64)
    pe[:, 0::2] = np.sin(i / 10000.0 ** ((2 * j) // E))
    pe[:, 1::2] = np.cos(i / 10000.0 ** ((2 * (j + 1)) // E))
    return pe.astype(np.float32)


def _build(with_bias, repeat=1, timing=False, ablate=()):
    """Build + compile the SPMD program. with_bias enables the rarely-needed
    bias adds (the reference generates all-zero biases). repeat>1 emits the
    whole computation repeat times back-to-back; timing=True makes all inputs
    device-Internal (garbage data) and shrinks the output so the RPC transfer
    floor vanishes (timing harness only)."""
    import concourse.bacc as bacc
    import concourse.mybir as mybir
    import concourse.tile as tile

    f32 = mybir.dt.float32
    bf16 = mybir.dt.bfloat16
    AF = mybir.ActivationFunctionType

    nc = bacc.Bacc("TRN2", target_bir_lowering=False, debug=False,
                   num_devices=N_CORES)

    def din(name, shape, dt):
        kind = "Internal" if timing else "ExternalInput"
        return nc.dram_tensor(name, list(shape), dt, kind=kind)

    xpt_d = din("xpt", (BPC, EC1, 128, S), f32)
    xpt16_d = din("xpt16", (BPC, EC1, 128, S), bf16)
    wq1_d = din("wq1", (EC1, 128, H * DK), bf16)
    wk1_d = din("wk1", (EC1, 128, H * DK), bf16)
    wv1_d = din("wv1", (EC1, 128, H * DK), bf16)
    wo1_d = din("wo1", (H, 128, D2), bf16)
    wff1_d = din("wff1", (DC, 128, D2), bf16)
    bff1_d = din("bff1", (128, DC), f32)
    wq2_d = din("wq2", (EC2, 128, H * DK), bf16)
    wk2_d = din("wk2", (EC2, 128, H * DK), bf16)
    wv2_d = din("wv2", (EC2, 128, H * DK), bf16)
    wo2_d = din("wo2", (DC, 128, H * DK), bf16)  # d-chunk-sliced layout
    wff2_d = din("wff2", (DC, 128, D2), bf16)
    bff2_d = din("bff2", (128, DC), f32)
    if with_bias:
        bqk1_d = din("bqk1", (2, 128, H), f32)   # [q/k][dk][head]
        bv1_d = din("bv1", (1, H * DK), f32)
        bo1_d = din("bo1", (128, DC), f32)
        bqk2_d = din("bqk2", (2, 128, H), f32)
        bv2_d = din("bv2", (1, H * DK), f32)
        bo2_d = din("bo2", (128, DC), f32)
    if timing:
        out_d = nc.dram_tensor("out_big", [BPC, D2, S], f32, kind="Internal")
        outsm_d = nc.dram_tensor("out", [128, 512], f32,
                                 kind="ExternalOutput")
    else:
        out_d = nc.dram_tensor("out", [BPC, D2, S], f32,
                               kind="ExternalOutput")

    import concourse.bass as bass
    from contextlib import ExitStack

    with tile.TileContext(nc) as tc:
        with ExitStack() as ctx:
            ep = ctx.enter_context
            p_y = ep(tc.tile_pool(name="p_y", bufs=1))
            p_stats = ep(tc.tile_pool(name="p_stats", bufs=1))

            p_e16 = ep(tc.tile_pool(name="p_e16", bufs=9))
            p_z16 = ep(tc.tile_pool(name="p_z16", bufs=8))
            p_kq = ep(tc.tile_pool(name="p_kq", bufs=6))
            p_vsl = ep(tc.tile_pool(name="p_vsl", bufs=8))
            p_v16 = ep(tc.tile_pool(name="p_v16", bufs=6))
            p_ms = ep(tc.tile_pool(name="p_ms", bufs=5))
            p_tanh = ep(tc.tile_pool(name="p_tanh", bufs=3))
            p_xh = ep(tc.tile_pool(name="p_xh", bufs=26))
            p_sqt = ep(tc.tile_pool(name="p_sqt", bufs=3))
            p_di = ep(tc.tile_pool(name="p_di", bufs=10))
            ps512 = ep(tc.tile_pool(name="ps512", bufs=8, space="PSUM"))
            p_dram = ep(tc.tile_pool(name="p_dram", bufs=2, space="DRAM"))

            y_slabs = [
                p_y.tile([128, DC * S], f32, tag=f"y{b}", name=f"y{b}")
                for b in range(BPC)
            ]
            sum_slab = p_stats.tile([128, DC * S], f32, tag="sum", name="sum")
            sq_slab = p_stats.tile([128, DC * S], f32, tag="sq", name="sq")
            eps_t = p_stats.tile([128, 1], f32, tag="eps", name="eps")
            nc.vector.memset(eps_t, float(EPS))
            f16 = mybir.dt.float16
            fp16_ar = "fp32ar" not in ablate
            if fp16_ar:
                # fp16 staging for the AllReduce payload (halves wire bytes;
                # fp16's 10-bit mantissa keeps the moments accurate, unlike
                # bf16)
                st_sum = p_stats.tile([128, DC * S // 2], f16, tag="stsum",
                                      name="stsum")
                st_sq = p_stats.tile([128, DC * S // 2], f16, tag="stsq",
                                     name="stsq")

            def cs(c):
                return slice(c * S, (c + 1) * S)

            def attention(b, x16h, wq, wk, wv, wo_lhsT, resid, y_slab, ECn,
                          bqk=None, bv_bc=None, bo=None):
                # x16h: [ECn][2] SBUF tiles [128, SH] bf16, feature-major,
                # split by s-half so projections can start on half 0 while
                # half 1's batch-norm AllReduce is still in flight.
                v16 = []
                for t in range(TCN):
                    xh = t // 2
                    xs = (t % 2) * 128
                    vp1 = ps512.tile([128, 512], f32, tag="ps", name="vp1")
                    vp2 = ps512.tile([128, 256], f32, tag="ps", name="vp2")
                    for ec in range(ECn):
                        nc.tensor.matmul(
                            vp1, lhsT=x16h[ec][xh][:, xs:xs + 128],
                            rhs=wv[:, ec, 0:512],
                            start=(ec == 0), stop=(ec == ECn - 1))
                    for ec in range(ECn):
                        nc.tensor.matmul(
                            vp2, lhsT=x16h[ec][xh][:, xs:xs + 128],
                            rhs=wv[:, ec, 512:768],
                            start=(ec == 0), stop=(ec == ECn - 1))
                    vt = p_v16.tile([128, H * DK], bf16, tag="v16",
                                    name=f"v16_{t}")
                    nc.vector.tensor_copy(out=vt[:, 0:512], in_=vp1)
                    nc.vector.tensor_copy(out=vt[:, 512:768], in_=vp2)
                    if bv_bc is not None:
                        nc.vector.tensor_add(out=vt, in0=vt, in1=bv_bc)
                    v16.append(vt)

                z16 = []
                for h in range(H):
                    hk = slice(h * DK, (h + 1) * DK)
                    kp = ps512.tile([128, 512], f32, tag="ps", name="kp")
                    qp = ps512.tile([128, 512], f32, tag="ps", name="qp")
                    for sh_ in (0, 1):
                        ssl = slice(sh_ * SH, (sh_ + 1) * SH)
                        for ec in range(ECn):
                            nc.tensor.matmul(
                                kp[:, ssl], lhsT=wk[:, ec, hk],
                                rhs=x16h[ec][sh_],
                                start=(ec == 0), stop=(ec == ECn - 1))
                        for ec in range(ECn):
                            nc.tensor.matmul(
                                qp[:, ssl], lhsT=wq[:, ec, hk],
                                rhs=x16h[ec][sh_],
                                start=(ec == 0), stop=(ec == ECn - 1))
                    k16 = p_kq.tile([128, S], bf16, tag="kq", name="k16")
                    q16 = p_kq.tile([128, S], bf16, tag="kq", name="q16")
                    if bqk is not None:
                        nc.vector.tensor_scalar_add(
                            out=k16, in0=kp, scalar1=bqk[1, :, h:h + 1])
                        nc.vector.tensor_scalar(
                            out=q16, in0=qp, scalar1=bqk[0, :, h:h + 1],
                            scalar2=SCALE, op0=mybir.AluOpType.add,
                            op1=mybir.AluOpType.mult)
                    else:
                        nc.vector.tensor_copy(out=k16, in_=kp)
                        nc.scalar.activation(out=q16, in_=qp, func=AF.Copy,
                                             scale=SCALE)
                    es = []
                    dsum4 = p_di.tile([128, TCN], f32, tag="dsum",
                                      name="dsum4")
                    for t in range(TCN):
                        sp = ps512.tile([128, 512], f32, tag="ps", name="sp")
                        nc.tensor.matmul(sp,
                                         lhsT=k16[:, t * 128:(t + 1) * 128],
                                         rhs=q16, start=True, stop=True)
                        e16 = p_e16.tile([128, S], bf16, tag="e16",
                                         name=f"e16_{t}")
                        nc.scalar.activation(out=e16, in_=sp, func=AF.Exp,
                                             accum_out=dsum4[:, t:t + 1])
                        es.append(e16)
                    dinv4 = p_di.tile([128, TCN], f32, tag="dinv",
                                      name="dinv4")
                    nc.vector.reciprocal(out=dinv4, in_=dsum4)
                    op_ = ps512.tile([128, 512], f32, tag="ps", name="op")
                    for t in range(TCN):
                        vsl = p_vsl.tile([128, DK], bf16, tag="vsl",
                                         name="vsl")
                        nc.vector.tensor_scalar_mul(out=vsl,
                                                    in0=v16[t][:, hk],
                                                    scalar1=dinv4[:, t:t + 1])
                        nc.tensor.matmul(op_, lhsT=vsl, rhs=es[t],
                                         start=(t == 0), stop=(t == TCN - 1))
                    zt = p_z16.tile([128, S], bf16, tag="z16", name=f"z16_{h}")
                    nc.scalar.activation(out=zt, in_=op_, func=AF.Copy)
                    z16.append(zt)

                for c in range(DC):
                    wp_ = ps512.tile([128, 512], f32, tag="ps", name="wp")
                    lhsT_list = wo_lhsT(c)
                    for h in range(H):
                        nc.tensor.matmul(wp_, lhsT=lhsT_list[h], rhs=z16[h],
                                         start=(h == 0), stop=(h == H - 1))
                    if bo is not None:
                        nc.vector.tensor_scalar_add(out=wp_, in0=wp_,
                                                    scalar1=bo[:, c:c + 1])
                    nc.vector.tensor_add(out=y_slab[:, cs(c)], in0=wp_,
                                         in1=resid(c))

            def make_n16h(b, tagpfx):
                # bf16 half-tiles of the normalized slab, per (chunk, s-half)
                n16h = []
                for c in range(DC):
                    pair = []
                    for hh in (0, 1):
                        t = p_xh.tile([128, SH], bf16, tag="x16h",
                                      name=f"{tagpfx}_{b}_{c}_{hh}")
                        o0 = c * S + hh * SH
                        nc.gpsimd.tensor_copy(out=t,
                                              in_=y_slabs[b][:, o0:o0 + SH])
                        pair.append(t)
                    n16h.append(pair)
                return n16h

            def ffn(b, wff, bff, half):
                # s-half-wise FFN: n16 slices, matmuls, tanh and residual add
                # all touch only this s-half, so half 0 runs while half 1's
                # batch-norm AllReduce is still in flight.
                n16 = []
                for ci in range(DC):
                    t = p_xh.tile([128, SH], bf16, tag="x16h",
                                  name=f"nh_{b}_{ci}")
                    o0 = ci * S + half * SH
                    nc.gpsimd.tensor_copy(out=t,
                                          in_=y_slabs[b][:, o0:o0 + SH])
                    n16.append(t)
                for co in range(DC):
                    fps = ps512.tile([128, SH], f32, tag="ps", name="fps")
                    for ci in range(DC):
                        nc.tensor.matmul(
                            fps, lhsT=wff[:, ci, co * 128:(co + 1) * 128],
                            rhs=n16[ci], start=(ci == 0), stop=(ci == DC - 1))
                    th = p_tanh.tile([128, SH], f32, tag="tanh", name="th")
                    nc.scalar.activation(out=th, in_=fps, func=AF.Tanh,
                                         bias=bff[:, co:co + 1], scale=1.0)
                    o0 = co * S + half * SH
                    ysl = y_slabs[b][:, o0:o0 + SH]
                    nc.vector.tensor_add(out=ysl, in0=th, in1=ysl)

            SH = S // 2                              # s-half width (256)

            def shalf(slab, h):
                # strided view of an [128, DC*S] slab: s-half h of every chunk
                return slab[:].rearrange("p (c s) -> p c s",
                                         c=DC)[:, :, h * SH:(h + 1) * SH]

            def bn(idx):
                # Stats are accumulated over the whole slab, then the
                # AllReduce + finalize + normalize are split over two
                # s-halves: everything after the first half's AllReduce
                # (normalize + the next phase's s-half-0 compute) overlaps
                # the second half's AllReduce.
                if "nobn" in ablate:
                    return
                for b in range(BPC):
                    if b == 0:
                        nc.vector.tensor_copy(out=sum_slab, in_=y_slabs[0])
                    else:
                        nc.vector.tensor_add(out=sum_slab, in0=sum_slab,
                                             in1=y_slabs[b])
                    for c in range(DC):
                        yc = y_slabs[b][:, cs(c)]
                        if b == 0:
                            nc.scalar.activation(out=sq_slab[:, cs(c)],
                                                 in_=yc, func=AF.Square)
                        else:
                            tmp = p_sqt.tile([128, S], f32, tag="sqt",
                                             name="sqt")
                            nc.scalar.activation(out=tmp, in_=yc,
                                                 func=AF.Square)
                            nc.vector.tensor_add(out=sq_slab[:, cs(c)],
                                                 in0=sq_slab[:, cs(c)],
                                                 in1=tmp)
                HS = DC * SH                         # per-half payload (2048)
                for half in (0, 1):
                    sums = shalf(sum_slab, half)
                    sqs = shalf(sq_slab, half)
                    ardt = f16 if fp16_ar else f32
                    arin = p_dram.tile([128, 2 * HS], ardt, tag=f"arin{half}",
                                       name=f"arin{idx}_{half}")
                    arout = p_dram.tile([128, 2 * HS], ardt,
                                        tag=f"arout{half}",
                                        name=f"arout{idx}_{half}",
                                        addr_space="Shared")
                    if fp16_ar:
                        stsum_v = st_sum[:].rearrange("p (c s) -> p c s",
                                                      c=DC)
                        stsq_v = st_sq[:].rearrange("p (c s) -> p c s", c=DC)
                        nc.vector.tensor_copy(out=stsum_v, in_=sums)
                        nc.vector.tensor_copy(out=stsq_v, in_=sqs)
                        src_sum, src_sq = st_sum[:], st_sq[:]
                    else:
                        src_sum, src_sq = sums, sqs
                    # chunked bounce DMAs parallelize across DMA queues
                    for i in range(2):
                        qn = HS // 2
                        if fp16_ar:
                            nc.sync.dma_start(
                                out=arin[:, i * qn:(i + 1) * qn],
                                in_=st_sum[:, i * qn:(i + 1) * qn])
                            nc.sync.dma_start(
                                out=arin[:, HS + i * qn:HS + (i + 1) * qn],
                                in_=st_sq[:, i * qn:(i + 1) * qn])
                        else:
                            csl = slice(i * DC // 2, (i + 1) * DC // 2)
                            nc.sync.dma_start(
                                out=arin[:, i * qn:(i + 1) * qn].rearrange(
                                    "p (c s) -> p c s", c=DC // 2),
                                in_=sums[:, csl, :])
                            nc.sync.dma_start(
                                out=arin[:, HS + i * qn:HS + (i + 1) * qn]
                                .rearrange("p (c s) -> p c s", c=DC // 2),
                                in_=sqs[:, csl, :])
                    if "noar" in ablate:
                        nc.sync.dma_start(out=arout, in_=arin)
                    else:
                        nc.gpsimd.collective_compute(
                            "AllReduce", mybir.AluOpType.add,
                            replica_groups=[list(range(N_CORES))],
                            ins=[arin.opt()], outs=[arout.opt()])
                    for i in range(2):
                        qn = HS // 2
                        if fp16_ar:
                            nc.sync.dma_start(
                                out=st_sum[:, i * qn:(i + 1) * qn],
                                in_=arout[:, i * qn:(i + 1) * qn])
                            nc.sync.dma_start(
                                out=st_sq[:, i * qn:(i + 1) * qn],
                                in_=arout[:, HS + i * qn:HS + (i + 1) * qn])
                        else:
                            csl = slice(i * DC // 2, (i + 1) * DC // 2)
                            nc.sync.dma_start(
                                out=sums[:, csl, :],
                                in_=arout[:, i * qn:(i + 1) * qn]
                                .rearrange("p (c s) -> p c s", c=DC // 2))
                            nc.sync.dma_start(
                                out=sqs[:, csl, :],
                                in_=arout[:, HS + i * qn:HS + (i + 1) * qn]
                                .rearrange("p (c s) -> p c s", c=DC // 2))
                    # finalize in place: sum half -> mean, sq half -> rstd
                    if fp16_ar:
                        nc.vector.tensor_scalar_mul(out=sums, in0=stsum_v,
                                                    scalar1=1.0 / B)
                        nc.vector.tensor_scalar_mul(out=sqs, in0=stsq_v,
                                                    scalar1=1.0 / B)
                    else:
                        nc.vector.tensor_scalar_mul(out=sums, in0=sums,
                                                    scalar1=1.0 / B)
                        nc.vector.tensor_scalar_mul(out=sqs, in0=sqs,
                                                    scalar1=1.0 / B)
                    for c in range(DC):
                        hs0 = c * S + half * SH
                        msq = p_ms.tile([128, SH], f32, tag="ms", name="msq")
                        nc.vector.tensor_mul(
                            out=msq, in0=sum_slab[:, hs0:hs0 + SH],
                            in1=sum_slab[:, hs0:hs0 + SH])
                        nc.vector.tensor_sub(
                            out=sq_slab[:, hs0:hs0 + SH],
                            in0=sq_slab[:, hs0:hs0 + SH], in1=msq)
                    nc.scalar.activation(out=sqs, in_=sqs, func=AF.Sqrt,
                                         bias=eps_t)
                    nc.vector.reciprocal(out=sqs, in_=sqs)
                    for b in range(BPC):
                        ysl = shalf(y_slabs[b], half)
                        nc.vector.tensor_sub(out=ysl, in0=ysl, in1=sums)
                        nc.vector.tensor_mul(out=ysl, in0=ysl, in1=sqs)

            def l1_attention():
                with tc.tile_pool(name="wl1", bufs=1) as wp, \
                     tc.tile_pool(name="p_xpf", bufs=5) as p_xpf:
                    wq = wp.tile([128, EC1, H * DK], bf16, name="wq1s")
                    wk = wp.tile([128, EC1, H * DK], bf16, name="wk1s")
                    wv = wp.tile([128, EC1, H * DK], bf16, name="wv1s")
                    wo = wp.tile([128, H, D2], bf16, name="wo1s")
                    nc.sync.dma_start(
                        out=wq, in_=wq1_d.ap().rearrange("c p k -> p c k"))
                    nc.sync.dma_start(
                        out=wk, in_=wk1_d.ap().rearrange("c p k -> p c k"))
                    nc.sync.dma_start(
                        out=wv, in_=wv1_d.ap().rearrange("c p k -> p c k"))
                    nc.sync.dma_start(
                        out=wo, in_=wo1_d.ap().rearrange("h p d -> p h d"))
                    bqk1 = bv1_bc = bo1 = None
                    if with_bias:
                        bqk1 = wp.tile([2, 128, H], f32, name="bqk1s")
                        nc.sync.dma_start(out=bqk1, in_=bqk1_d.ap())
                        bv1t = wp.tile([1, H * DK], f32, name="bv1s")
                        nc.sync.dma_start(out=bv1t, in_=bv1_d.ap())
                        bv1_bc = bass.AP(
                            tensor=bv1t.tensor, offset=bv1t.offset,
                            ap=[[0, 128], [1, H * DK]])
                        bo1 = wp.tile([128, DC], f32, name="bo1s")
                        nc.sync.dma_start(out=bo1, in_=bo1_d.ap())

                    def wo_lhsT1(c):
                        return [wo[:, h, c * 128:(c + 1) * 128]
                                for h in range(H)]

                    for b in range(BPC):
                        x16h = []
                        xf4 = []
                        for c in range(EC1):
                            pair = []
                            for hh in (0, 1):
                                t = p_xh.tile([128, SH], bf16, tag="x16h",
                                              name=f"x16_{b}_{c}_{hh}")
                                nc.sync.dma_start(
                                    out=t,
                                    in_=xpt16_d.ap()[b, c][:, hh * SH:
                                                           (hh + 1) * SH])
                                pair.append(t)
                            x16h.append(pair)
                            xf = p_xpf.tile([128, S], f32, tag="xpf",
                                            name=f"xpf_{c}")
                            nc.sync.dma_start(out=xf, in_=xpt_d.ap()[b, c])
                            xf4.append(xf)

                        def resid1(c, xf4=xf4):
                            return xf4[c % EC1]

                        attention(b, x16h, wq, wk, wv, wo_lhsT1, resid1,
                                  y_slabs[b], EC1, bqk=bqk1, bv_bc=bv1_bc,
                                  bo=bo1)

            def ffn_phase(wff_d, bff_d, pname):
                with tc.tile_pool(name=pname, bufs=1) as wp:
                    wff = wp.tile([128, DC, D2], bf16, name=f"{pname}w")
                    nc.sync.dma_start(
                        out=wff, in_=wff_d.ap().rearrange("c p d -> p c d"))
                    bff = wp.tile([128, DC], f32, name=f"{pname}b")
                    nc.sync.dma_start(out=bff, in_=bff_d.ap())
                    for half in (0, 1):
                        for b in range(BPC):
                            ffn(b, wff, bff, half)

            def l2_attention():
                with tc.tile_pool(name="wl2", bufs=1) as wp, \
                     tc.tile_pool(name="wo2p", bufs=3) as wo2p:
                    wq2 = wp.tile([128, EC2, H * DK], bf16, name="wq2s")
                    wk2 = wp.tile([128, EC2, H * DK], bf16, name="wk2s")
                    wv2 = wp.tile([128, EC2, H * DK], bf16, name="wv2s")
                    nc.sync.dma_start(
                        out=wq2, in_=wq2_d.ap().rearrange("c p k -> p c k"))
                    nc.sync.dma_start(
                        out=wk2, in_=wk2_d.ap().rearrange("c p k -> p c k"))
                    nc.sync.dma_start(
                        out=wv2, in_=wv2_d.ap().rearrange("c p k -> p c k"))
                    bqk2 = bv2_bc = bo2 = None
                    if with_bias:
                        bqk2 = wp.tile([2, 128, H], f32, name="bqk2s")
                        nc.sync.dma_start(out=bqk2, in_=bqk2_d.ap())
                        bv2t = wp.tile([1, H * DK], f32, name="bv2s")
                        nc.sync.dma_start(out=bv2t, in_=bv2_d.ap())
                        bv2_bc = bass.AP(
                            tensor=bv2t.tensor, offset=bv2t.offset,
                            ap=[[0, 128], [1, H * DK]])
                        bo2 = wp.tile([128, DC], f32, name="bo2s")
                        nc.sync.dma_start(out=bo2, in_=bo2_d.ap())

                    for b in range(BPC):
                        n16h = make_n16h(b, "na")

                        def wo_lhsT2(c):
                            w = wo2p.tile([128, H * DK], bf16, tag="wo2s",
                                          name="wo2s")
                            nc.sync.dma_start(out=w, in_=wo2_d.ap()[c])
                            return [w[:, h * DK:(h + 1) * DK]
                                    for h in range(H)]

                        def resid2(c, b=b):
                            return y_slabs[b][:, cs(c)]

                        attention(b, n16h, wq2, wk2, wv2, wo_lhsT2, resid2,
                                  y_slabs[b], EC2, bqk=bqk2, bv_bc=bv2_bc,
                                  bo=bo2)

            for _rep in range(repeat):
                if "noattn" not in ablate:
                    l1_attention()
                bn(0)
                if "noffn" not in ablate:
                    ffn_phase(wff1_d, bff1_d, "wf1")
                bn(1)
                if "noattn" not in ablate:
                    l2_attention()
                bn(2)
                if "noffn" not in ablate:
                    ffn_phase(wff2_d, bff2_d, "wf2")
                bn(3)

            # ---------------- Output ----------------
            for half in (0, 1):
                for b in range(BPC):
                    nc.sync.dma_start(
                        out=out_d.ap()[b].rearrange("(c p) s -> p c s",
                                                    p=128)
                        [:, :, half * SH:(half + 1) * SH],
                        in_=shalf(y_slabs[b], half))
            if timing:
                nc.sync.dma_start(out=outsm_d.ap(),
                                  in_=y_slabs[0][:, 0:512])

    nc.compile()
    return nc


def _get_program(with_bias):
    key = bool(with_bias)
    if key not in _PROGRAM_CACHE:
        _PROGRAM_CACHE[key] = _build(key)
    return _PROGRAM_CACHE[key]


def _pack_qkv(w):
    # (H, din, DK) -> (din//128, 128, H*DK)
    din = w.shape[1]
    return np.ascontiguousarray(
        w.transpose(1, 0, 2).reshape(din, H * DK).reshape(din // 128, 128,
                                                          H * DK))


def _prep_in_maps(inputs):
    bf = ml_dtypes.bfloat16
    f32 = np.float32
    g = lambda n: np.asarray(inputs[n], f32)

    X = g("X")
    pos = _pos_encoding()
    xp = (X + pos[None]).astype(f32)                       # (B, S, E)
    xpT = np.ascontiguousarray(xp.transpose(0, 2, 1))      # (B, E, S)
    xpt = xpT.reshape(B, EC1, 128, S)
    xpt16 = xpt.astype(bf)

    wq1 = _pack_qkv(g("Wq1")).astype(bf)
    wk1 = _pack_qkv(g("Wk1")).astype(bf)
    wv1 = _pack_qkv(g("Wv1")).astype(bf)
    wo1 = np.ascontiguousarray(g("Wo1").reshape(H, 128, D2)).astype(bf)
    wff1 = np.ascontiguousarray(g("Wff1").reshape(DC, 128, D2)).astype(bf)
    bff1 = np.ascontiguousarray(g("bff1").reshape(DC, 128).T).astype(f32)
    wq2 = _pack_qkv(g("Wq2")).astype(bf)
    wk2 = _pack_qkv(g("Wk2")).astype(bf)
    wv2 = _pack_qkv(g("Wv2")).astype(bf)
    # Wo2 sliced by output d-chunk: wo2[c, p, h*DK+m] = Wo2[h*DK+p, c*128+m]
    wo2 = np.ascontiguousarray(
        g("Wo2").reshape(H, DK, DC, 128).transpose(2, 1, 0, 3)
        .reshape(DC, 128, H * DK)).astype(bf)
    wff2 = np.ascontiguousarray(g("Wff2").reshape(DC, 128, D2)).astype(bf)
    bff2 = np.ascontiguousarray(g("bff2").reshape(DC, 128).T).astype(f32)

    zb = [g(n) for n in ("bq1", "bk1", "bv1", "bo1", "bq2", "bk2", "bv2",
                         "bo2")]
    with_bias = any(np.any(z) for z in zb)

    shared = dict(wq1=wq1, wk1=wk1, wv1=wv1, wo1=wo1, wff1=wff1, bff1=bff1,
                  wq2=wq2, wk2=wk2, wv2=wv2, wo2=wo2, wff2=wff2, bff2=bff2)
    if with_bias:
        bq1, bk1, bv1, bo1, bq2, bk2, bv2, bo2 = zb
        shared.update(
            bqk1=np.stack([bq1.T, bk1.T]).astype(f32),
            bv1=bv1.reshape(1, H * DK).astype(f32),
            bo1=np.ascontiguousarray(bo1.reshape(DC, 128).T).astype(f32),
            bqk2=np.stack([bq2.T, bk2.T]).astype(f32),
            bv2=bv2.reshape(1, H * DK).astype(f32),
            bo2=np.ascontiguousarray(bo2.reshape(DC, 128).T).astype(f32),
        )

    in_maps = []
    for core in range(N_CORES):
        m = dict(shared)
        m["xpt"] = np.ascontiguousarray(xpt[core * BPC:(core + 1) * BPC])
        m["xpt16"] = np.ascontiguousarray(xpt16[core * BPC:(core + 1) * BPC])
        in_maps.append(m)
    return in_maps, with_bias


def kernel(**inputs):
    in_maps, with_bias = _prep_in_maps(inputs)
    nc = _get_program(with_bias)
    from concourse import bass_utils
    res = bass_utils.run_bass_kernel_spmd(nc, in_maps,
                                          core_ids=list(range(N_CORES)))
    outT = np.concatenate([res.results[i]["out"] for i in range(N_CORES)],
                          axis=0)                          # (B, D2, S)
    return np.ascontiguousarray(outT.transpose(0, 2, 1))   # (B, S, D2) f32
